# revision 22
# baseline (speedup 1.0000x reference)
"""Trainium2 Bass kernel for ConstrainedAttentionModel (sparse_attention).

Full-input contract: kernel(x=[8,2048] int, C=[4,4] f32) -> [8,2048] f32.
Data parallel across 8 NeuronCores: one batch row per core.

Math (per row, T=2048, k=4, V=2048):
  scores[t] = sum_{i,j} C[i,j] * [x[t-j] == x[T-1-i]]   (t-j >= 0)
  scores[T-1] = -1e9; attn = softmax(scores)
  out[v] = sum_t attn[t] * [x[t] == v]

Device strategy (t = 16p + f layout on 128 partitions):
  - all operands arrive pre-converted to fp16 (token ids < 2048 are
    exact in fp16), so the DVE pipeline needs zero prep casts:
      sync DMA:   19-token window per partition (shifted lag-j copies
                  are overlapping SBUF views, stride -1 on j)
      scalar DMA: per-token lo = x&63 and hi = x-lo
      vector DMA: q (replicated) then C (replicated) - tiny stride-0
                  reads, much earlier than a PE-broadcast round trip
      gpsimd:     lo/hi iota rows (on-device constants)
  - equality/score chain in fp16 (packed 2x DVE mode), single fused
    reduce + single exp -> E fp16
  - vocab one-hot factorized as v = 64*hi + lo: A[hi,f]=E[f]*[x>>6==hi]
    (transposed so the E-multiply hits 2x mode; lhsT tolerates the
    stride), B[f,lo]=[x&63==lo] (contiguous: matmul rhs must stream
    packed); out[hi,lo] = 16 PSUM-accumulated fp16 matmuls
  - the t=T-1 (softmax-masked) position is excluded by contracting only
    127 partitions in the last matmul
  - the kernel returns the UNNORMALIZED histogram; softmax
    normalization happens on host: out = y / y.sum() (Z == sum(y))
"""
import os
import numpy as np
import concourse.bass as bass
import concourse.bacc as bacc
import concourse.tile as tile
from concourse import mybir

P = 128
T = 2048
F = T // P  # 16
K = 4
FH = F // 2  # 8
NHI = 32
NLO = 64
XW = F + K - 1  # 19

fp32 = mybir.dt.float32
fp16 = mybir.dt.float16
Alu = mybir.AluOpType
Act = mybir.ActivationFunctionType

N_WARM1 = int(os.environ.get("KERNEL_N_WARM1", "14"))
N_WARM2 = int(os.environ.get("KERNEL_N_WARM2", "22"))

B = 8


def _build_nc():
    nc = bacc.Bacc()
    xwin = nc.dram_tensor("xwin", [K - 1 + T], fp16, kind="ExternalInput")
    xlh = nc.dram_tensor("xlh", [P * 2 * F], fp16, kind="ExternalInput")
    qc = nc.dram_tensor("qc", [32], fp16, kind="ExternalInput")
    y = nc.dram_tensor("y", [T], fp32, kind="ExternalOutput")

    with tile.TileContext(nc) as tc:
        with (
            tc.tile_pool(name="sb", bufs=1) as sb,
            tc.tile_pool(name="ps", bufs=1, space="PSUM") as ps,
        ):
            XF16 = sb.tile([P, XW], fp16)  # XF16[p,e] = x[16p+e-3], pad -1
            XLOHI = sb.tile([P, 2 * F], fp16)  # [lo(16) | hi(16)]
            QR = sb.tile([P, 16], fp16)  # q[i] at 4i+j, replicated
            CC = sb.tile([P, 16], fp16)  # C[i,j] at 4i+j, replicated

            nc.sync.dma_start(
                out=XF16[:],
                in_=bass.AP(tensor=xwin[:].tensor, offset=0, ap=[[F, P], [1, XW]]),
            )
            nc.scalar.dma_start(
                out=QR[:],
                in_=bass.AP(tensor=qc[:].tensor, offset=0, ap=[[0, P], [1, 16]]),
            )
            nc.scalar.dma_start(
                out=CC[:],
                in_=bass.AP(tensor=qc[:].tensor, offset=16, ap=[[0, P], [1, 16]]),
            )
            nc.scalar.dma_start(
                out=XLOHI[:],
                in_=bass.AP(
                    tensor=xlh[:].tensor, offset=0, ap=[[2 * F, P], [1, 2 * F]]
                ),
            )
            XLO = XLOHI[:, 0:F]
            XHI = XLOHI[:, F : 2 * F]

            # on-device iota rows for the one-hots (tiny, finish early)
            IL16 = sb.tile([P, NLO], fp16)
            IH16 = sb.tile([P, NHI], fp16)
            nc.gpsimd.iota(
                IL16[:], pattern=[[1, NLO]], channel_multiplier=0,
                allow_small_or_imprecise_dtypes=True,
            )
            nc.gpsimd.iota(
                IH16[:], pattern=[[64, NHI]], channel_multiplier=0,
                allow_small_or_imprecise_dtypes=True,
            )

            c1 = nc.const_aps.aps[(fp32, 1.0)]

            # PE warm-up: narrow matmuls keep the HAM clock gate open
            warm = ps.tile([1, 1], fp32)
            for w in range(N_WARM1 + N_WARM2):
                nc.tensor.matmul(
                    warm[:], lhsT=c1[:, 0:1], rhs=c1[:, 0:1], start=True,
                    stop=True, skip_group_check=True,
                )

            Q16 = QR[:].rearrange("p (i j) -> p i j", j=K)

            EQ = sb.tile([P, F, K, K], fp16)
            CE = sb.tile([P, F, 16], fp16)
            SC = sb.tile([P, F], fp16)
            E = sb.tile([P, F], fp16)
            AEQ = sb.tile([P, NHI, F], fp16)  # transposed: [hi, f]
            Bt = sb.tile([P, F, NLO], fp16)
            A = sb.tile([P, NHI, F], fp16)  # transposed: lhsT slice per f
            acc = ps.tile([NHI, NLO], fp32)

            # EQ[p,f,i,j] = [x[t-j] == q_i]  (t = 16p+f)
            sub = XF16[:, K - 1 :][:]
            XWIN = bass.AP(
                tensor=sub.tensor,
                offset=sub.offset,
                ap=[sub.ap[0], [1, F], [0, K], [-1, K]],
            )
            with tc.high_priority():
                nc.vector.tensor_tensor(
                    out=EQ[:],
                    in0=XWIN,
                    in1=Q16[:, None, :, :].broadcast_to([P, F, K, K]),
                    op=Alu.is_equal,
                )
                nc.vector.tensor_tensor(
                    out=CE[:],
                    in0=EQ[:].rearrange("p f i j -> p f (i j)"),
                    in1=CC[:, None, :].broadcast_to([P, F, 16]),
                    op=Alu.mult,
                )
                with nc.allow_low_precision(reason="16 products of |C|<0.1"):
                    nc.vector.reduce_sum(
                        out=SC[:], in_=CE[:], axis=mybir.AxisListType.X
                    )
                nc.scalar.activation(out=E[:], in_=SC[:], func=Act.Exp)

            for h in range(2):
                fs = slice(h * FH, (h + 1) * FH)
                nc.vector.tensor_tensor(
                    out=Bt[:, fs],
                    in0=XLO[:, fs, None].broadcast_to([P, FH, NLO]),
                    in1=IL16[:, None, :].broadcast_to([P, FH, NLO]),
                    op=Alu.is_equal,
                )
                nc.vector.tensor_tensor(
                    out=AEQ[:, :, fs],
                    in0=XHI[:, None, fs].broadcast_to([P, NHI, FH]),
                    in1=IH16[:, :, None].broadcast_to([P, NHI, FH]),
                    op=Alu.is_equal,
                )
                nc.vector.tensor_tensor(
                    out=A[:, :, fs],
                    in0=AEQ[:, :, fs],
                    in1=E[:, None, fs].broadcast_to([P, NHI, FH]),
                    op=Alu.mult,
                )
                for f in range(h * FH, (h + 1) * FH):
                    # t=2047 (p=127, f=15) is excluded from the contraction
                    # entirely -> attn[T-1] = 0 and Z skips it
                    pe = P - 1 if f == F - 1 else P
                    nc.tensor.matmul(
                        acc[:],
                        lhsT=A[0:pe, :, f],
                        rhs=Bt[0:pe, f, :],
                        start=(f == 0),
                        stop=(f == F - 1),
                        skip_group_check=True,
                    )

            OUT = sb.tile([NHI, NLO], fp32)
            nc.vector.tensor_copy(out=OUT[:], in_=acc[:])
            yv = y[:].rearrange("(h l) -> h l", l=NLO)
            nc.sync.dma_start(out=yv[0:16], in_=OUT[0:16, :])
            nc.scalar.dma_start(out=yv[16:32], in_=OUT[16:32, :])
    nc.compile()
    return nc


def _host_prep(x_row: np.ndarray, C: np.ndarray):
    x_row = x_row.astype(np.int32)
    xwin = np.concatenate(
        [np.full(K - 1, -1, np.float16), x_row.astype(np.float16)]
    )
    lo = (x_row & 63).astype(np.float16)
    hi = (x_row - (x_row & 63)).astype(np.float16)
    xlh = np.concatenate(
        [lo.reshape(P, F), hi.reshape(P, F)], axis=1
    ).reshape(-1)
    q = x_row[T - 1 : T - 1 - K : -1].astype(np.float16)  # q[i] = x[T-1-i]
    qc = np.concatenate(
        [np.repeat(q, K), C.reshape(16).astype(np.float16)]
    )
    return {"xwin": xwin, "xlh": xlh, "qc": qc}


_NC_CACHE = {}


def _get_nc():
    if "nc" not in _NC_CACHE:
        _NC_CACHE["nc"] = _build_nc()
    return _NC_CACHE["nc"]


def kernel(x: np.ndarray, C: np.ndarray, _spmd_kwargs: dict | None = None):
    from concourse.bass_utils import run_bass_kernel_spmd

    x = np.asarray(x).astype(np.int32)  # token ids < 2048, exact
    C = np.asarray(C).astype(np.float32)
    assert x.shape == (B, T) and C.shape == (K, K)
    in_maps = [_host_prep(x[b], C) for b in range(B)]
    res = run_bass_kernel_spmd(
        _get_nc(), in_maps, core_ids=list(range(B)), **(_spmd_kwargs or {})
    )
    # y is the unnormalized E-weighted vocab histogram; Z == y.sum()
    hist = np.stack([res.results[b]["y"] for b in range(B)], axis=0)
    out = (hist / hist.sum(axis=1, keepdims=True)).astype(np.float32)
    if _spmd_kwargs:
        kernel.last_results = res
    return out
